# revision 10
# baseline (speedup 1.0000x reference)
"""Trainium2 kernel for nn_MultiHeadCrossAttention_28063316313030.

Math: with seq_len == 1, softmax over a size-1 axis is identically 1, so
attention(Q,K,V) == V and W_Q/W_K are dead code.  The whole module collapses to

    out = LN(x1 @ A) + LN(x2 @ A),   A = W_V.T @ W_fc.T   (1024 x 1024)

where LN is LayerNorm over the last dim (gamma/beta fold in on host).

Distribution: pure data parallel over the batch dim across 8 NeuronCores.
Host precomputes A (tiny matmul) and pre-tiles x1/x2 C-major so the TensorE
contraction dim lands on SBUF partitions with fully contiguous DMA runs.

Everything on the PE path is bf16 (x, A, and the stored output, which the
host upcasts to f32): the moving operand streams at ~218ns per 512-row
matmul vs ~233ns for fp32r, and DMA bytes halve.  LayerNorm stays in f32
(PSUM + stats).  Measured rel err ~4e-3 vs the 2e-2 gate.

Schedule per core (2048 rows per stream, 16 b-tiles x 2 streams):
  PE warmup matmuls run on a memset tile (no DMA dependency) so the clock
  ramp starts right after sequencer 'main'; they end about when the first
  A k-tile lands.
  Startup DMA priority: A (2MB) is the critical stream - it goes first on
  gpsimd (even k) + scalar (odd k).  x b-tile 0 rides sync; b-tiles 1-2
  queue on gpsimd BEHIND the A tiles so they don't steal HBM bandwidth
  from A.  Later x tiles prefetch 2 deep on sync, naturally paced by the
  3-buffer x pool's WAR dependency.  Output stores issue on gpsimd (sync
  must never block, or x prefetch stalls).
  b-tiles run k-major across the 4 (stream, half) PSUM banks: each A
  k-tile feeds 4 matmuls, keeping the PE near the A stream's arrival rate.
  Fused epilogue per b-tile: bn_stats/bn_aggr -> mean/var per stream,
  r = 1/sqrt(var+eps) (ACT sqrt + DVE recip), nmr = -mu*r.  Stream-0
  normalizes via ACT Identity with bias = nmr0+nmr1; stream-1 fuses
  normalize + cross-stream add in ONE DVE pass:
      out = (ps1 * r1) + n0'   (scalar_tensor_tensor, bf16 out).
  The last two b-tiles reorder work so the final b-tile's critical stats
  sit ahead of bt14's normalize in the DVE FIFO; the final stream
  normalizes on ACT (parallel engine) into bf16, adds at 2x DVE rate, and
  splits its last store across gpsimd+sync.
"""

import sys

sys.path.insert(0, "/opt/trn_rl_repo")

import numpy as np

B, C, OUT = 16384, 1024, 1024
EPS = 1e-5
NCORES = 8
R = B // NCORES  # rows per core per stream
P = 128
KT = C // P  # contraction tiles
BT = R // P  # row tiles per core
NH = OUT // 512  # psum bank halves per row tile
N_WARMUP = 4

_cache = {}


def _build(mm_dtype_name: str, out_dtype_name: str):
    import concourse.bacc as bacc
    import concourse.bass as bass
    import concourse.mybir as mybir
    from concourse.tile import TileContext

    f32 = mybir.dt.float32
    bf16 = mybir.dt.bfloat16
    mmdt = getattr(mybir.dt, mm_dtype_name)
    outdt = getattr(mybir.dt, out_dtype_name)
    AF = mybir.ActivationFunctionType
    ALU = mybir.AluOpType

    nc = bacc.Bacc("TRN2", target_bir_lowering=False, debug=False, num_devices=NCORES)

    # host-pretiled: [ki, bt, ko, bi]
    x1p = nc.declare_dram_parameter("x1p", [P, BT, KT, P], mmdt, isOutput=False)
    x2p = nc.declare_dram_parameter("x2p", [P, BT, KT, P], mmdt, isOutput=False)
    # host-pretiled: [ki, ko, o]
    a_d = nc.declare_dram_parameter("a", [P, KT, OUT], mmdt, isOutput=False)
    y_d = nc.declare_dram_parameter("y", [R, OUT], outdt, isOutput=True)

    with TileContext(nc) as tc:
        with (
            tc.tile_pool(name="singles", bufs=1) as singles,
            tc.tile_pool(name="xs", bufs=3) as xpool,
            tc.tile_pool(name="ns", bufs=3) as npool,
            tc.tile_pool(name="outs", bufs=3) as opool,
            tc.tile_pool(name="stats", bufs=4) as stats,
            tc.tile_pool(name="psum", bufs=2, space="PSUM") as psum,
        ):
            # --- PE warmup on a memset tile: no DMA dependency.
            warm_sb = singles.tile([P, 512], bf16)
            nc.vector.memset(warm_sb, 0.5)
            warm_ps = psum.tile([P, 512], f32, tag="ps11")
            for w in range(N_WARMUP):
                lo = 128 * (w % 2)
                nc.tensor.matmul(
                    warm_ps[:], lhsT=warm_sb[:, lo : lo + P], rhs=warm_sb[:],
                    start=True, stop=True,
                )

            # --- A k-tiles first on gpsimd/scalar (the critical stream).
            a_sb = [None] * KT
            for k in range(KT):
                t = singles.tile([P, OUT], mmdt, tag=f"a{k}", name=f"a{k}")
                eng = nc.gpsimd if k % 2 == 0 else nc.scalar
                eng.dma_start(t[:], a_d[:, k, :])
                a_sb[k] = t

            # x b-tile 0 on sync (needed first); 1-2 queue behind A on gpsimd.
            xt_pre = {}

            def issue_x(bt, eng):
                for s, xp in enumerate((x1p, x2p)):
                    t = xpool.tile(
                        [P, KT, P], mmdt, tag=f"xt{s}", name=f"xt{bt}_{s}"
                    )
                    eng.dma_start(t[:], xp[:, bt])
                    xt_pre[(bt, s)] = t

            issue_x(0, nc.sync)
            issue_x(1, nc.gpsimd)
            issue_x(2, nc.gpsimd)

            eps_sb = singles.tile([P, 1], f32)
            nc.vector.memset(eps_sb, EPS)

            def stream_stats(bt, s, ps_tiles):
                """bn stats -> r = 1/sqrt(var+eps), nmr = -mu*r for one stream."""
                st = stats.tile([P, NH, 6], f32, tag=f"st{s}", name=f"st{bt}{s}")
                for h in range(NH):
                    nc.vector.bn_stats(st[:, h, :], ps_tiles[h][:])
                mv = stats.tile([P, 2], f32, tag=f"mv{s}", name=f"mv{bt}{s}")
                nc.vector.bn_aggr(mv[:], st[:])
                r_sb = stats.tile([P, 1], f32, tag=f"r{s}", name=f"r{bt}{s}")
                nc.scalar.activation(
                    r_sb[:], mv[:, 1:2], func=AF.Sqrt, bias=eps_sb[:], scale=1.0
                )
                nc.vector.reciprocal(r_sb[:], r_sb[:])
                nmr = stats.tile([P, 1], f32, tag=f"nmr{s}", name=f"nmr{bt}{s}")
                nc.vector.tensor_scalar(
                    nmr[:],
                    mv[:, 0:1],
                    scalar1=r_sb[:],
                    scalar2=-1.0,
                    op0=ALU.mult,
                    op1=ALU.mult,
                )
                return r_sb, nmr

            def make_ps(bt):
                return {
                    s: [
                        psum.tile(
                            [P, 512], f32, tag=f"ps{s}{h}", name=f"ps{bt}{s}{h}"
                        )
                        for h in range(NH)
                    ]
                    for s in range(2)
                }

            def mm(ps_bt, xts, s, h, k):
                nc.tensor.matmul(
                    ps_bt[s][h][:],
                    lhsT=xts[s][:, k, :],
                    rhs=a_sb[k][:, h * 512 : (h + 1) * 512],
                    start=(k == 0),
                    stop=(k == KT - 1),
                )

            def finish(bt, ps_bt, rn0, rn1):
                """n0' = ps0*r0 + (nmr0+nmr1) on ACT, then one DVE pass:
                out = ps1*r1 + n0' (bf16), one store on gpsimd."""
                r0, nmr0 = rn0
                r1, nmr1 = rn1
                nmrs = stats.tile([P, 1], f32, tag="nmrs", name=f"nmrs{bt}")
                nc.vector.tensor_tensor(nmrs[:], nmr0[:], nmr1[:], op=ALU.add)
                ntile = npool.tile([P, OUT], f32, tag="n0", name=f"n{bt}")
                out_t = opool.tile([P, OUT], outdt, tag="out", name=f"out{bt}")
                for h in range(NH):
                    sl = slice(h * 512, (h + 1) * 512)
                    nc.scalar.activation(
                        ntile[:, sl], ps_bt[0][h][:],
                        func=AF.Identity, bias=nmrs[:], scale=r0[:],
                    )
                    nc.vector.scalar_tensor_tensor(
                        out_t[:, sl], ps_bt[1][h][:], r1[:], ntile[:, sl],
                        op0=ALU.mult, op1=ALU.add,
                    )
                nc.gpsimd.dma_start(y_d[bt * P : (bt + 1) * P, :], out_t[:])

            # --- prologue pair: b-tiles 0+1 interleave k0..k3 (8 matmuls
            # per scarce early A k-tile, matching the A DMA arrival rate),
            # then finish their k4..k7 sequentially once A is resident.
            xts0 = {s: xt_pre[(0, s)] for s in range(2)}
            xts1 = {s: xt_pre[(1, s)] for s in range(2)}
            ps_b0 = make_ps(0)
            ps_b1 = make_ps(1)
            for k in range(KT // 2):
                for ps_bt, xts in ((ps_b0, xts0), (ps_b1, xts1)):
                    for s in range(2):
                        for h in range(NH):
                            mm(ps_bt, xts, s, h, k)
            for k in range(KT // 2, KT):
                for s in range(2):
                    for h in range(NH):
                        mm(ps_b0, xts0, s, h, k)
            rn0 = stream_stats(0, 0, ps_b0[0])
            rn1 = stream_stats(0, 1, ps_b0[1])
            finish(0, ps_b0, rn0, rn1)
            for k in range(KT // 2, KT):
                for s in range(2):
                    for h in range(NH):
                        mm(ps_b1, xts1, s, h, k)
            rn0 = stream_stats(1, 0, ps_b1[0])
            rn1 = stream_stats(1, 1, ps_b1[1])
            finish(1, ps_b1, rn0, rn1)
            issue_x(3, nc.sync)

            # --- steady b-tiles 2..BT-3
            for bt in range(2, BT - 2):
                if bt + 2 < BT and (bt + 2, 0) not in xt_pre:
                    issue_x(bt + 2, nc.sync)
                xts = {s: xt_pre[(bt, s)] for s in range(2)}
                ps_bt = make_ps(bt)
                for k in range(KT):
                    for s in range(2):
                        for h in range(NH):
                            mm(ps_bt, xts, s, h, k)
                rn0 = stream_stats(bt, 0, ps_bt[0])
                rn1 = stream_stats(bt, 1, ps_bt[1])
                finish(bt, ps_bt, rn0, rn1)

            # --- last two b-tiles: bt15's critical stats ahead of bt14's
            # normalize in the DVE FIFO; final stream normalizes on ACT
            # into bf16, adds at 2x DVE rate, split last store.
            b14, b15 = BT - 2, BT - 1
            xts14 = {s: xt_pre[(b14, s)] for s in range(2)}
            xts15 = {s: xt_pre[(b15, s)] for s in range(2)}
            ps14 = make_ps(b14)
            ps15 = make_ps(b15)
            for k in range(KT):
                for s in range(2):
                    for h in range(NH):
                        mm(ps14, xts14, s, h, k)
            rn14_0 = stream_stats(b14, 0, ps14[0])
            rn14_1 = stream_stats(b14, 1, ps14[1])
            for k in range(KT):
                for h in range(NH):
                    mm(ps15, xts15, 0, h, k)
            rn15_0 = stream_stats(b15, 0, ps15[0])
            # bt14 finish via ACT-normalize (both streams) + 2x-rate bf16
            # ADD: keeps heavy DVE work out of bt15's critical FIFO window.
            n14a = npool.tile([P, OUT], bf16, tag="n14a", name="n14a")
            n14b = npool.tile([P, OUT], bf16, tag="n14b", name="n14b")
            out14 = opool.tile([P, OUT], outdt, tag="out", name=f"out{b14}")
            for h in range(NH):
                sl = slice(h * 512, (h + 1) * 512)
                nc.scalar.activation(
                    n14a[:, sl], ps14[0][h][:],
                    func=AF.Identity, bias=rn14_0[1][:], scale=rn14_0[0][:],
                )
                nc.scalar.activation(
                    n14b[:, sl], ps14[1][h][:],
                    func=AF.Identity, bias=rn14_1[1][:], scale=rn14_1[0][:],
                )
                nc.vector.tensor_tensor(
                    out14[:, sl], n14a[:, sl], n14b[:, sl], op=ALU.add
                )
            nc.gpsimd.dma_start(y_d[b14 * P : (b14 + 1) * P, :], out14[:])
            # bt15 s0 normalize off the critical path (during s1 matmuls)
            r0, nmr0 = rn15_0
            n15 = npool.tile([P, OUT], bf16, tag="n0b", name=f"n{b15}")
            for h in range(NH):
                sl = slice(h * 512, (h + 1) * 512)
                nc.scalar.activation(
                    n15[:, sl], ps15[0][h][:],
                    func=AF.Identity, bias=nmr0[:], scale=r0[:],
                )
            for h in range(NH):
                for k in range(KT):
                    mm(ps15, xts15, 1, h, k)
            r1, nmr1 = stream_stats(b15, 1, ps15[1])
            out_t = opool.tile([P, OUT], outdt, tag="out", name=f"out{b15}")
            n1t = npool.tile([P, OUT], bf16, tag="n1b", name="n1t")
            for h in range(NH):
                sl = slice(h * 512, (h + 1) * 512)
                nc.scalar.activation(
                    n1t[:, sl], ps15[1][h][:],
                    func=AF.Identity, bias=nmr1[:], scale=r1[:],
                )
                nc.vector.tensor_tensor(
                    out_t[:, sl], n1t[:, sl], n15[:, sl], op=ALU.add
                )
                if h == 0:
                    nc.gpsimd.dma_start(
                        y_d[b15 * P : (b15 + 1) * P, 0:512], out_t[:, 0:512]
                    )
                else:
                    nc.sync.dma_start(
                        y_d[b15 * P : (b15 + 1) * P, 512:768],
                        out_t[:, 512:768],
                    )
                    nc.gpsimd.dma_start(
                        y_d[b15 * P : (b15 + 1) * P, 768:1024],
                        out_t[:, 768:1024],
                    )

    nc.finalize()
    return nc


def _get_nc(mm_dtype_name: str, out_dtype_name: str):
    key = (mm_dtype_name, out_dtype_name)
    if key not in _cache:
        _cache[key] = _build(mm_dtype_name, out_dtype_name)
    return _cache[key]


def _pretile_x(x_core: np.ndarray) -> np.ndarray:
    # [R, C] -> [ki, bt, ko, bi]
    return np.ascontiguousarray(
        x_core.reshape(BT, P, KT, P).transpose(3, 0, 2, 1)
    )


def kernel(x1, x2, W_Q, W_K, W_V, W_fc, gamma, beta, _trace=False,
           _mm_dtype="bfloat16", _out_dtype="bfloat16"):
    from concourse.bass_utils import run_bass_kernel_spmd

    x1 = np.asarray(x1, dtype=np.float32)
    x2 = np.asarray(x2, dtype=np.float32)
    W_V = np.asarray(W_V, dtype=np.float32)
    W_fc = np.asarray(W_fc, dtype=np.float32)
    gamma = np.asarray(gamma, dtype=np.float32)
    beta = np.asarray(beta, dtype=np.float32)

    # A = W_V.T @ W_fc.T in float64 to keep the host collapse error negligible.
    A = (W_V.T.astype(np.float64) @ W_fc.T.astype(np.float64)).astype(np.float32)
    # [C, OUT] -> [ki, ko, o]
    Ap = np.ascontiguousarray(A.reshape(KT, P, OUT).transpose(1, 0, 2))

    # Device computes LN(y1)+LN(y2); any affine LN params fold in on host:
    # out = (LN1+LN2)*gamma + 2*beta.  (This problem has gamma=1, beta=0.)
    use_affine = not (np.all(gamma == 1.0) and np.all(beta == 0.0))

    if _mm_dtype == "bfloat16":
        import ml_dtypes

        np_mm = ml_dtypes.bfloat16
    else:
        np_mm = np.float32
    Ap = Ap.astype(np_mm)

    in_maps = []
    for r in range(NCORES):
        sl = slice(r * R, (r + 1) * R)
        m = {
            "x1p": _pretile_x(x1[sl]).astype(np_mm),
            "x2p": _pretile_x(x2[sl]).astype(np_mm),
            "a": Ap,
        }
        in_maps.append(m)

    nc = _get_nc(_mm_dtype, _out_dtype)
    res = run_bass_kernel_spmd(nc, in_maps, list(range(NCORES)), trace=_trace)

    y = np.concatenate(
        [np.asarray(res.results[r]["y"], dtype=np.float32) for r in range(NCORES)],
        axis=0,
    )
    if use_affine:
        y = y * gamma[None, :] + 2.0 * beta[None, :]
    out = y.reshape(B, 1, OUT)
    if _trace:
        return out, res
    return out


# revision 11
# speedup vs baseline: 1.0179x; 1.0179x over previous
"""Trainium2 kernel for nn_MultiHeadCrossAttention_28063316313030.

Math: with seq_len == 1, softmax over a size-1 axis is identically 1, so
attention(Q,K,V) == V and W_Q/W_K are dead code.  The whole module collapses to

    out = LN(x1 @ A) + LN(x2 @ A),   A = W_V.T @ W_fc.T   (1024 x 1024)

where LN is LayerNorm over the last dim (gamma/beta fold in on host).

Distribution: pure data parallel over the batch dim across 8 NeuronCores.
Host precomputes A (tiny matmul) and pre-tiles x1/x2 C-major so the TensorE
contraction dim lands on SBUF partitions with fully contiguous DMA runs.

Everything on the PE path is bf16 (x, A, and the stored output, which the
host upcasts to f32): the moving operand streams at ~218ns per 512-row
matmul vs ~233ns for fp32r, and DMA bytes halve.  LayerNorm stays in f32
(PSUM + stats).  Measured rel err ~4e-3 vs the 2e-2 gate.

Schedule per core (2048 rows per stream, 16 b-tiles x 2 streams):
  PE warmup matmuls run on a memset tile (no DMA dependency) so the clock
  ramp starts right after sequencer 'main'; they end about when the first
  A k-tile lands.
  Startup DMA priority: A (2MB) is the critical stream - it goes first on
  gpsimd (even k) + scalar (odd k).  x b-tile 0 rides sync; b-tiles 1-2
  queue on gpsimd BEHIND the A tiles so they don't steal HBM bandwidth
  from A.  Later x tiles prefetch 2 deep on sync, naturally paced by the
  3-buffer x pool's WAR dependency.  Output stores issue on gpsimd (sync
  must never block, or x prefetch stalls).
  b-tiles run k-major across the 4 (stream, half) PSUM banks: each A
  k-tile feeds 4 matmuls, keeping the PE near the A stream's arrival rate.
  Fused epilogue per b-tile: bn_stats/bn_aggr -> mean/var per stream,
  r = 1/sqrt(var+eps) (ACT sqrt + DVE recip), nmr = -mu*r.  Stream-0
  normalizes via ACT Identity with bias = nmr0+nmr1; stream-1 fuses
  normalize + cross-stream add in ONE DVE pass:
      out = (ps1 * r1) + n0'   (scalar_tensor_tensor, bf16 out).
  The last two b-tiles reorder work so the final b-tile's critical stats
  sit ahead of bt14's normalize in the DVE FIFO; the final stream
  normalizes on ACT (parallel engine) into bf16, adds at 2x DVE rate, and
  splits its last store across gpsimd+sync.
"""

import sys

sys.path.insert(0, "/opt/trn_rl_repo")

import numpy as np

B, C, OUT = 16384, 1024, 1024
EPS = 1e-5
NCORES = 8
R = B // NCORES  # rows per core per stream
P = 128
KT = C // P  # contraction tiles
BT = R // P  # row tiles per core
NH = OUT // 512  # psum bank halves per row tile
N_WARMUP = 4

_cache = {}


def _build(mm_dtype_name: str, out_dtype_name: str):
    import concourse.bacc as bacc
    import concourse.bass as bass
    import concourse.mybir as mybir
    from concourse.tile import TileContext

    f32 = mybir.dt.float32
    bf16 = mybir.dt.bfloat16
    mmdt = getattr(mybir.dt, mm_dtype_name)
    outdt = getattr(mybir.dt, out_dtype_name)
    AF = mybir.ActivationFunctionType
    ALU = mybir.AluOpType

    nc = bacc.Bacc("TRN2", target_bir_lowering=False, debug=False, num_devices=NCORES)

    # host-pretiled: [ki, bt, ko, bi]
    x1p = nc.declare_dram_parameter("x1p", [P, BT, KT, P], mmdt, isOutput=False)
    x2p = nc.declare_dram_parameter("x2p", [P, BT, KT, P], mmdt, isOutput=False)
    # host-pretiled: [ki, ko, o]
    a_d = nc.declare_dram_parameter("a", [P, KT, OUT], mmdt, isOutput=False)
    y_d = nc.declare_dram_parameter("y", [R, OUT], outdt, isOutput=True)

    with TileContext(nc) as tc:
        with (
            tc.tile_pool(name="singles", bufs=1) as singles,
            tc.tile_pool(name="xs", bufs=3) as xpool,
            tc.tile_pool(name="ns", bufs=3) as npool,
            tc.tile_pool(name="outs", bufs=3) as opool,
            tc.tile_pool(name="stats", bufs=4) as stats,
            tc.tile_pool(name="psum", bufs=2, space="PSUM") as psum,
        ):
            # --- PE warmup on a memset tile: no DMA dependency.
            warm_sb = singles.tile([P, 512], bf16)
            nc.vector.memset(warm_sb, 0.5)
            warm_ps = psum.tile([P, 512], f32, tag="ps11")
            for w in range(N_WARMUP):
                lo = 128 * (w % 2)
                nc.tensor.matmul(
                    warm_ps[:], lhsT=warm_sb[:, lo : lo + P], rhs=warm_sb[:],
                    start=True, stop=True,
                )

            # --- A k-tiles first on gpsimd/scalar (the critical stream).
            a_sb = [None] * KT
            for k in range(KT):
                t = singles.tile([P, OUT], mmdt, tag=f"a{k}", name=f"a{k}")
                eng = nc.gpsimd if k % 2 == 0 else nc.scalar
                eng.dma_start(t[:], a_d[:, k, :])
                a_sb[k] = t

            # x b-tile 0 on sync (needed first); 1-2 queue behind A on gpsimd.
            xt_pre = {}

            def issue_x(bt, eng):
                for s, xp in enumerate((x1p, x2p)):
                    t = xpool.tile(
                        [P, KT, P], mmdt, tag=f"xt{s}", name=f"xt{bt}_{s}"
                    )
                    eng.dma_start(t[:], xp[:, bt])
                    xt_pre[(bt, s)] = t

            issue_x(0, nc.sync)
            issue_x(1, nc.gpsimd)
            issue_x(2, nc.gpsimd)

            eps_sb = singles.tile([P, 1], f32)
            nc.vector.memset(eps_sb, EPS)

            def stream_stats(bt, s, ps_tiles):
                """bn stats -> r = 1/sqrt(var+eps), nmr = -mu*r for one stream."""
                st = stats.tile([P, NH, 6], f32, tag=f"st{s}", name=f"st{bt}{s}")
                for h in range(NH):
                    nc.vector.bn_stats(st[:, h, :], ps_tiles[h][:])
                mv = stats.tile([P, 2], f32, tag=f"mv{s}", name=f"mv{bt}{s}")
                nc.vector.bn_aggr(mv[:], st[:])
                r_sb = stats.tile([P, 1], f32, tag=f"r{s}", name=f"r{bt}{s}")
                nc.scalar.activation(
                    r_sb[:], mv[:, 1:2], func=AF.Sqrt, bias=eps_sb[:], scale=1.0
                )
                nc.vector.reciprocal(r_sb[:], r_sb[:])
                nmr = stats.tile([P, 1], f32, tag=f"nmr{s}", name=f"nmr{bt}{s}")
                nc.vector.tensor_scalar(
                    nmr[:],
                    mv[:, 0:1],
                    scalar1=r_sb[:],
                    scalar2=-1.0,
                    op0=ALU.mult,
                    op1=ALU.mult,
                )
                return r_sb, nmr

            def make_ps(bt):
                return {
                    s: [
                        psum.tile(
                            [P, 512], f32, tag=f"ps{s}{h}", name=f"ps{bt}{s}{h}"
                        )
                        for h in range(NH)
                    ]
                    for s in range(2)
                }

            def mm(ps_bt, xts, s, h, k):
                nc.tensor.matmul(
                    ps_bt[s][h][:],
                    lhsT=xts[s][:, k, :],
                    rhs=a_sb[k][:, h * 512 : (h + 1) * 512],
                    start=(k == 0),
                    stop=(k == KT - 1),
                )

            def finish(bt, ps_bt, rn0, rn1):
                """n0' = ps0*r0 + (nmr0+nmr1) on ACT, then one DVE pass:
                out = ps1*r1 + n0' (bf16), one store on gpsimd."""
                r0, nmr0 = rn0
                r1, nmr1 = rn1
                nmrs = stats.tile([P, 1], f32, tag="nmrs", name=f"nmrs{bt}")
                nc.vector.tensor_tensor(nmrs[:], nmr0[:], nmr1[:], op=ALU.add)
                ntile = npool.tile([P, OUT], f32, tag="n0", name=f"n{bt}")
                out_t = opool.tile([P, OUT], outdt, tag="out", name=f"out{bt}")
                for h in range(NH):
                    sl = slice(h * 512, (h + 1) * 512)
                    nc.scalar.activation(
                        ntile[:, sl], ps_bt[0][h][:],
                        func=AF.Identity, bias=nmrs[:], scale=r0[:],
                    )
                    nc.vector.scalar_tensor_tensor(
                        out_t[:, sl], ps_bt[1][h][:], r1[:], ntile[:, sl],
                        op0=ALU.mult, op1=ALU.add,
                    )
                nc.gpsimd.dma_start(y_d[bt * P : (bt + 1) * P, :], out_t[:])

            # --- steady b-tiles 0..BT-3
            for bt in range(BT - 2):
                if bt + 2 < BT and (bt + 2, 0) not in xt_pre:
                    issue_x(bt + 2, nc.sync)
                xts = {s: xt_pre[(bt, s)] for s in range(2)}
                ps_bt = make_ps(bt)
                for k in range(KT):
                    for s in range(2):
                        for h in range(NH):
                            mm(ps_bt, xts, s, h, k)
                rn0 = stream_stats(bt, 0, ps_bt[0])
                rn1 = stream_stats(bt, 1, ps_bt[1])
                if bt == BT - 3:
                    break  # bt13: stats done; finish emitted after bt15-s0
                finish(bt, ps_bt, rn0, rn1)

            # --- endgame: hoist bt15's stream-0 (matmuls + stats +
            # normalize) ahead of bt14, so the final window holds only
            # bt15-s1 and the post-last-matmul chain is minimal.
            b13, b14, b15 = BT - 3, BT - 2, BT - 1
            xts15 = {s2: xt_pre[(b15, s2)] for s2 in range(2)}
            ps15_0 = [
                psum.tile([P, 512], f32, tag=f"ps0{h}", name=f"ps{b15}0{h}")
                for h in range(NH)
            ]
            for k in range(KT):
                for h in range(NH):
                    nc.tensor.matmul(
                        ps15_0[h][:],
                        lhsT=xts15[0][:, k, :],
                        rhs=a_sb[k][:, h * 512 : (h + 1) * 512],
                        start=(k == 0),
                        stop=(k == KT - 1),
                    )
            finish(b13, ps_bt, rn0, rn1)
            rn15_0 = stream_stats(b15, 0, ps15_0)
            n15 = npool.tile([P, OUT], bf16, tag="n0b", name=f"n{b15}")
            for h in range(NH):
                sl = slice(h * 512, (h + 1) * 512)
                nc.scalar.activation(
                    n15[:, sl], ps15_0[h][:],
                    func=AF.Identity, bias=rn15_0[1][:], scale=rn15_0[0][:],
                )

            # --- bt14 (ACT-normalize both streams + 2x-rate bf16 add,
            # keeping DVE light), then bt15-s1 as the minimal final unit.
            xts14 = {s2: xt_pre[(b14, s2)] for s2 in range(2)}
            ps14 = make_ps(b14)
            for k in range(KT):
                for s in range(2):
                    for h in range(NH):
                        mm(ps14, xts14, s, h, k)
            rn14_0 = stream_stats(b14, 0, ps14[0])
            rn14_1 = stream_stats(b14, 1, ps14[1])
            n14a = npool.tile([P, OUT], bf16, tag="n14a", name="n14a")
            n14b = npool.tile([P, OUT], bf16, tag="n14b", name="n14b")
            out14 = opool.tile([P, OUT], outdt, tag="out", name=f"out{b14}")
            for h in range(NH):
                sl = slice(h * 512, (h + 1) * 512)
                nc.scalar.activation(
                    n14a[:, sl], ps14[0][h][:],
                    func=AF.Identity, bias=rn14_0[1][:], scale=rn14_0[0][:],
                )
                nc.scalar.activation(
                    n14b[:, sl], ps14[1][h][:],
                    func=AF.Identity, bias=rn14_1[1][:], scale=rn14_1[0][:],
                )
            ps15_1 = [
                psum.tile([P, 512], f32, tag=f"ps1{h}", name=f"ps{b15}1{h}")
                for h in range(NH)
            ]
            for h in range(NH):
                for k in range(KT):
                    nc.tensor.matmul(
                        ps15_1[h][:],
                        lhsT=xts15[1][:, k, :],
                        rhs=a_sb[k][:, h * 512 : (h + 1) * 512],
                        start=(k == 0),
                        stop=(k == KT - 1),
                    )
            for h in range(NH):
                sl = slice(h * 512, (h + 1) * 512)
                nc.vector.tensor_tensor(
                    out14[:, sl], n14a[:, sl], n14b[:, sl], op=ALU.add
                )
            nc.gpsimd.dma_start(y_d[b14 * P : (b14 + 1) * P, :], out14[:])
            r1, nmr1 = stream_stats(b15, 1, ps15_1)
            out15 = opool.tile([P, OUT], outdt, tag="out", name=f"out{b15}")
            n1t = npool.tile([P, OUT], bf16, tag="n1b", name="n1t")
            for h in range(NH):
                sl = slice(h * 512, (h + 1) * 512)
                nc.scalar.activation(
                    n1t[:, sl], ps15_1[h][:],
                    func=AF.Identity, bias=nmr1[:], scale=r1[:],
                )
                nc.vector.tensor_tensor(
                    out15[:, sl], n1t[:, sl], n15[:, sl], op=ALU.add
                )
                eng = nc.gpsimd if h == 0 else nc.sync
                eng.dma_start(
                    y_d[b15 * P : (b15 + 1) * P, sl], out15[:, sl]
                )

    nc.finalize()
    return nc


def _get_nc(mm_dtype_name: str, out_dtype_name: str):
    key = (mm_dtype_name, out_dtype_name)
    if key not in _cache:
        _cache[key] = _build(mm_dtype_name, out_dtype_name)
    return _cache[key]


def _pretile_x(x_core: np.ndarray) -> np.ndarray:
    # [R, C] -> [ki, bt, ko, bi]
    return np.ascontiguousarray(
        x_core.reshape(BT, P, KT, P).transpose(3, 0, 2, 1)
    )


def kernel(x1, x2, W_Q, W_K, W_V, W_fc, gamma, beta, _trace=False,
           _mm_dtype="bfloat16", _out_dtype="bfloat16"):
    from concourse.bass_utils import run_bass_kernel_spmd

    x1 = np.asarray(x1, dtype=np.float32)
    x2 = np.asarray(x2, dtype=np.float32)
    W_V = np.asarray(W_V, dtype=np.float32)
    W_fc = np.asarray(W_fc, dtype=np.float32)
    gamma = np.asarray(gamma, dtype=np.float32)
    beta = np.asarray(beta, dtype=np.float32)

    # A = W_V.T @ W_fc.T in float64 to keep the host collapse error negligible.
    A = (W_V.T.astype(np.float64) @ W_fc.T.astype(np.float64)).astype(np.float32)
    # [C, OUT] -> [ki, ko, o]
    Ap = np.ascontiguousarray(A.reshape(KT, P, OUT).transpose(1, 0, 2))

    # Device computes LN(y1)+LN(y2); any affine LN params fold in on host:
    # out = (LN1+LN2)*gamma + 2*beta.  (This problem has gamma=1, beta=0.)
    use_affine = not (np.all(gamma == 1.0) and np.all(beta == 0.0))

    if _mm_dtype == "bfloat16":
        import ml_dtypes

        np_mm = ml_dtypes.bfloat16
    else:
        np_mm = np.float32
    Ap = Ap.astype(np_mm)

    in_maps = []
    for r in range(NCORES):
        sl = slice(r * R, (r + 1) * R)
        m = {
            "x1p": _pretile_x(x1[sl]).astype(np_mm),
            "x2p": _pretile_x(x2[sl]).astype(np_mm),
            "a": Ap,
        }
        in_maps.append(m)

    nc = _get_nc(_mm_dtype, _out_dtype)
    res = run_bass_kernel_spmd(nc, in_maps, list(range(NCORES)), trace=_trace)

    y = np.concatenate(
        [np.asarray(res.results[r]["y"], dtype=np.float32) for r in range(NCORES)],
        axis=0,
    )
    if use_affine:
        y = y * gamma[None, :] + 2.0 * beta[None, :]
    out = y.reshape(B, 1, OUT)
    if _trace:
        return out, res
    return out
